# revision 9
# baseline (speedup 1.0000x reference)
"""Trainium2 Bass kernel for the 2D-LSTM (nn_Lstm2D) problem.

Reference computation (B=64, C=3, H=W=128, P=4 patch, NC=512 cells):
  - image is cut into a 32x32 grid of 4x4 patches, raster-scanned (1024 steps)
  - per step t=(i,j):  gates = [x_t, h_prevrow_j] @ W_ih.T + h_{t-1} @ W_hh.T + b
                       i,f,g,o = split(gates); c = sig(f)*c + sig(i)*tanh(g)
                       h = sig(o)*tanh(c)
  - output: h at every grid cell -> (B, 512, 32, 32)

Strategy (8 NeuronCores, data-parallel over batch, 8 batch elements/core):
  - x-contribution and previous-row contribution of the gates are batched per
    row into one PSUM accumulation (PRE), leaving only h @ W_hh.T + elementwise
    on the sequential critical path.
  - recurrence matmul is weight-stationary: lhsT = W_hh.T tile [128,128] (bf16,
    gets fast-weight-load), rhs = h^T [128, 8], output gates^T in PSUM with the
    gate dimension on partitions -- the layout the LSTM elementwise needs.
  - elementwise on ScalarE (sigmoid/tanh) + VectorE (mul/add), fp32 state.
  - one hardware For_i loop over the 32 rows; 32 steps unrolled in the body.

All tensor reshapes/transposes are done host-side in numpy; the device sees
layouts with the partition dim outermost.
"""

import numpy as np
import ml_dtypes

B = 64
C = 3
H = W = 128
P = 4
NCELL = 512
IN = C * P * P          # 48
SY = SX = 32
NCORES = 8
BL = B // NCORES        # 8 batch elements per core
KC = NCELL // 128       # 4 contraction chunks for h
MC = (4 * NCELL) // 128  # 16 gate-dim chunks
# psum/gates column-slot order: (i, f, o, g) blocks of 4 cell-chunks each, so
# sigmoid covers slots 0..11 in one op and tanh(g) covers 12..15.
SLOT_TO_MCHUNK = [0, 1, 2, 3, 4, 5, 6, 7, 12, 13, 14, 15, 8, 9, 10, 11]

BF16 = ml_dtypes.bfloat16


def _build_module(sy=SY):
    import concourse.bass as bass
    import concourse.bacc as bacc
    import concourse.tile as tile
    import concourse.mybir as mybir

    f32 = mybir.dt.float32
    bf16 = mybir.dt.bfloat16

    nc = bacc.Bacc()

    x_d = nc.declare_dram_parameter("xt", [IN, sy, SX * BL], bf16, isOutput=False)
    whh_d = nc.declare_dram_parameter("whht", [128, KC * MC * 128], bf16, isOutput=False)
    wp_d = nc.declare_dram_parameter("wpt", [128, KC * MC * 128], bf16, isOutput=False)
    wx_d = nc.declare_dram_parameter("wxt", [IN, MC * 128], bf16, isOutput=False)
    bias_d = nc.declare_dram_parameter("biast", [128, MC], f32, isOutput=False)
    out_d = nc.declare_dram_parameter("out", [128, KC, sy * SX, BL], f32, isOutput=True)

    with tile.TileContext(nc) as tc:
        with (
            tc.tile_pool(name="persist", bufs=1) as persist,
            tc.tile_pool(name="rowbuf", bufs=2) as rowbuf,
            tc.tile_pool(name="gates", bufs=3) as gpool,
            tc.tile_pool(name="tmp", bufs=4) as tpool,
            tc.tile_pool(name="psg", bufs=2, space="PSUM") as psg_pool,
            tc.tile_pool(name="psp", bufs=2, space="PSUM") as psp_pool,
        ):
            whh_sb = persist.tile([128, KC, MC, 128], bf16)
            wp_sb = persist.tile([128, KC, MC, 128], bf16)
            wx_sb = persist.tile([IN, MC, 128], bf16)
            bias_sb = persist.tile([128, MC], f32)
            c_sb = persist.tile([128, KC, BL], f32)
            hbf_cur = persist.tile([128, KC, SX, BL], bf16)
            hbf_prev = persist.tile([128, KC, SX, BL], bf16)

            nc.sync.dma_start(out=whh_sb[:], in_=whh_d[:])
            nc.sync.dma_start(out=wp_sb[:], in_=wp_d[:])
            nc.sync.dma_start(out=wx_sb[:], in_=wx_d[:])
            nc.sync.dma_start(out=bias_sb[:], in_=bias_d[:])
            nc.vector.memset(c_sb[:], 0.0)
            nc.vector.memset(hbf_cur[:], 0.0)
            nc.vector.memset(hbf_prev[:], 0.0)

            # pull the sigmoid/tanh ACT table load out of the row loop
            warm = persist.tile([1, 1], f32)
            nc.vector.memset(warm[:], 0.0)
            nc.scalar.activation(out=warm[:], in_=warm[:],
                                 func=mybir.ActivationFunctionType.Sigmoid)
            nc.scalar.activation(out=warm[:], in_=warm[:],
                                 func=mybir.ActivationFunctionType.Tanh)

            with tc.For_i(0, sy) as iv:
                # row-above h becomes "prev"; row 0 uses the initial zeros
                nc.vector.tensor_copy(out=hbf_prev[:], in_=hbf_cur[:])

                xrow = rowbuf.tile([IN, SX * BL], bf16)
                nc.gpsimd.dma_start(out=xrow[:], in_=x_d[:, bass.ds(iv, 1), :])

                # PRE[slot, j, b] = bias + x @ Wx.T + prevrow @ Wp.T
                # weights/bias are already slot-permuted host-side
                pre_sb = rowbuf.tile([128, MC, SX, BL], f32)
                for s in range(MC):
                    ps = psp_pool.tile([128, SX * BL], mybir.dt.float32)
                    nc.tensor.matmul(ps[:], wx_sb[:, s, :], xrow[:],
                                     start=True, stop=False)
                    for k in range(KC):
                        nc.tensor.matmul(
                            ps[:], wp_sb[:, k, s, :],
                            hbf_prev[:, k, :, :],
                            start=False, stop=(k == KC - 1))
                    nc.vector.tensor_scalar_add(
                        out=pre_sb[:, s, :, :], in0=ps[:],
                        scalar1=bias_sb[:, s:s + 1])

                hrow = rowbuf.tile([128, KC, SX, BL], mybir.dt.float32)
                for j in range(SX):
                    psg = psg_pool.tile([128, MC, BL], mybir.dt.float32)
                    for s in range(MC):
                        for k in range(KC):
                            rhs = (hbf_prev[:, k, SX - 1, :] if j == 0
                                   else hbf_cur[:, k, j - 1, :])
                            nc.tensor.matmul(
                                psg[:, s, :], whh_sb[:, k, s, :], rhs,
                                start=(k == 0), stop=(k == KC - 1))

                    gates = gpool.tile([128, MC, BL], mybir.dt.float32)
                    nc.vector.tensor_add(gates[:], psg[:], pre_sb[:, :, j, :])

                    acts = gpool.tile([128, MC, BL], mybir.dt.float32)
                    nc.scalar.activation(
                        out=acts[:, 0:12, :], in_=gates[:, 0:12, :],
                        func=mybir.ActivationFunctionType.Sigmoid)
                    nc.scalar.activation(
                        out=acts[:, 12:16, :], in_=gates[:, 12:16, :],
                        func=mybir.ActivationFunctionType.Tanh)
                    i_s = acts[:, 0:4, :]
                    f_s = acts[:, 4:8, :]
                    o_s = acts[:, 8:12, :]
                    g_t = acts[:, 12:16, :]

                    ig = tpool.tile([128, KC, BL], mybir.dt.float32)
                    nc.vector.tensor_mul(ig[:], i_s, g_t)
                    nc.vector.tensor_mul(c_sb[:], f_s, c_sb[:])
                    nc.vector.tensor_add(c_sb[:], c_sb[:], ig[:])

                    tc_t = tpool.tile([128, KC, BL], mybir.dt.float32)
                    nc.scalar.activation(
                        out=tc_t[:], in_=c_sb[:],
                        func=mybir.ActivationFunctionType.Tanh)

                    nc.vector.tensor_mul(hrow[:, :, j, :], o_s, tc_t[:])
                    nc.vector.tensor_copy(out=hbf_cur[:, :, j, :],
                                          in_=hrow[:, :, j, :])

                nc.gpsimd.dma_start(
                    out=out_d[:, :, bass.ds(iv * SX, SX), :], in_=hrow[:])

    nc.compile()
    return nc


_CACHE = {}


def _get_module(sy=SY):
    if sy not in _CACHE:
        _CACHE[sy] = _build_module(sy)
    return _CACHE[sy]


def _prep_shared(W_ih, W_hh, b_ih, b_hh):
    perm = np.array(SLOT_TO_MCHUNK)
    wih_t = np.ascontiguousarray(W_ih.T.astype(np.float32))     # (560, 2048)
    wx = wih_t[:IN]                                             # (48, 2048)
    wx = wx.reshape(IN, MC, 128)[:, perm, :].reshape(IN, MC * 128)
    wp = wih_t[IN:]                                             # (512, 2048)
    wp = wp.reshape(KC, 128, MC, 128)[:, :, perm, :]
    wp = wp.transpose(1, 0, 2, 3).reshape(128, KC * MC * 128)
    whh = np.ascontiguousarray(W_hh.T.astype(np.float32))       # (512, 2048)
    whh = whh.reshape(KC, 128, MC, 128)[:, :, perm, :]
    whh = whh.transpose(1, 0, 2, 3).reshape(128, KC * MC * 128)
    bias = (b_ih + b_hh).astype(np.float32).reshape(MC, 128)[perm]
    bias = np.ascontiguousarray(bias.T)                         # (128, 16)
    return (wx.astype(BF16), wp.astype(BF16), whh.astype(BF16), bias)


def _prep_x(batch, sy=SY):
    # xs[i, j, b, :] = patch (C,P,P) flattened, matching the reference
    xs = batch.reshape(B, C, sy, P, SX, P).transpose(2, 4, 0, 1, 3, 5)
    xs = xs.reshape(sy, SX, B, IN)
    per_core = []
    for c in range(NCORES):
        xc = xs[:, :, c * BL:(c + 1) * BL, :]          # (sy, SX, BL, IN)
        xc = xc.transpose(3, 0, 1, 2).reshape(IN, sy, SX * BL)
        per_core.append(np.ascontiguousarray(xc).astype(BF16))
    return per_core


def _run(batch, W_ih, W_hh, b_ih, b_hh, trace=False):
    from concourse.bass_utils import run_bass_kernel_spmd

    batch = np.asarray(batch, dtype=np.float32)
    wx, wp, whh, bias = _prep_shared(
        np.asarray(W_ih), np.asarray(W_hh), np.asarray(b_ih), np.asarray(b_hh))
    xs = _prep_x(batch)

    nc = _get_module()
    in_maps = [
        {"xt": xs[c], "whht": whh, "wpt": wp, "wxt": wx, "biast": bias}
        for c in range(NCORES)
    ]
    res = run_bass_kernel_spmd(nc, in_maps, list(range(NCORES)), trace=trace)

    outs = []
    for c in range(NCORES):
        arr = res.results[c]["out"]          # (128, KC, sy*SX, BL)
        # reference's to_image is a raw reshape of (B, T, NC) into
        # (B, NC, SY, SX): arr axes (BL, T, KC, 128) flatten to (BL, T*NC).
        arr = arr.transpose(3, 2, 1, 0).reshape(BL, NCELL, SY, SX)
        outs.append(arr)
    return np.concatenate(outs, axis=0).astype(np.float32), res


def kernel(batch, W_ih, W_hh, b_ih, b_hh):
    out, _ = _run(batch, W_ih, W_hh, b_ih, b_hh)
    return out
